# revision 36
# baseline (speedup 1.0000x reference)
"""GCN encoder (GCNConv + PReLU) as a Bass/Tile kernel on 8 Trainium2 NeuronCores.

Math (PyG GCNConv with self-loops + symmetric norm, then PReLU):
    deg[i] = in-degree over dst (+1 self loop); dinv = 1/sqrt(deg)
    agg[d] = dinv[d] * ( sum_{e:(s->d)} dinv[s]*x[s] + dinv[d]*x[d] )
    out    = PReLU(agg @ W.T + bias)

Distribution: dst-node sharding with a balanced permutation — dsts are
assigned to (core, block) cells greedily by edge count so every block needs
exactly ceil(~1020/128)=8 chunks on every core (0.35% padding); the host
unpermutes the output.

Key structure:
  - dinv[src] is folded into the features on the host (xp = dinv * x, bf16);
    dinv[dst] is applied on-chip as a per-partition scale during the
    PSUM->SBUF copy of the aggregate. Edge messages therefore need NO
    per-edge weight: the selection matrices are pure 0/1.
  - each core gathers from its OWN renumbered table of unique src rows
    (~31.6k < 32767, so a single int16-indexed dma_gather table, no halves).
    Edges are grouped by dst-block (128 dsts) and packed into 128-edge
    chunks; chunk counts are maxed over cores so all cores share a program.
  - gathers are merged: consecutive blocks are grouped until ~GRP chunks per
    dma_gather, rotated over 4 SWDGE queues.
  - per dst-block, ONE batched DVE op builds all selection matrices:
    ms[e, g*128+d] = (iota[d] == dstl[e, g]) via a stride-0 broadcast AP,
    all bf16. One PE matmul per chunk accumulates A[d,c] += ms_g^T @ gx_g.
  - the self-loop term adds as a dense identity matmul of xp rows.
  - A (f32 PSUM) is copied to bf16 with scale dinv[d], transposed on the PE
    (bf16 identity), then H = A^T W^T + bias accumulates in PSUM (bf16 in,
    f32 accum). PReLU = max(H, alpha*H) for 0<=alpha<=1, general fallback
    relu(H)*(1-alpha) + alpha*H.
"""

import os
import numpy as np
from contextlib import ExitStack

import concourse.tile as tile
from concourse import bacc, mybir, bass_utils

# Problem shape (fixed by the harness contract).
N_NODES = 50000
N_EDGES = 400000
IN_CH = 256
HID = 512
NCORES = 8
NPC = N_NODES // NCORES  # dst nodes owned per core
P = 128
TAB = 32768              # gather table rows (unique srcs per core, padded)

F32 = mybir.dt.float32
BF16 = mybir.dt.bfloat16
# target chunks per merged dma_gather instruction
GRP = int(os.environ.get("GCN_GRP", "18"))
# number of SWDGE queues to rotate gathers over
NQ = int(os.environ.get("GCN_NQ", "4"))


def _preprocess(edge_index, n_nodes=N_NODES, ncores=NCORES):
    """Per-core edge packing with renumbered unique-src gather tables.

    Returns (kblk, groups, idx16, dstl, uniq, dinv):
      kblk:   [bpc] chunks per dst-block (compile-time, maxed over cores)
      groups: list of lists of consecutive block ids, ~GRP chunks per group
      idx16:  [ncores, 128, 8*tot] int16 gather indices (16-wrap, 8x tiled)
      dstl:   [ncores, 128, tot] f32 dst-local-in-block per edge slot (-1 pad)
      uniq:   per-core sorted unique src ids (table content order)
      dinv:   [n_nodes] f32 1/sqrt(deg)
    """
    dblk = P
    npc = n_nodes // ncores
    src = np.asarray(edge_index[0]).astype(np.int64).ravel()
    dst = np.asarray(edge_index[1]).astype(np.int64).ravel()
    ecnt = np.bincount(dst, minlength=n_nodes)
    deg = ecnt.astype(np.float32) + 1.0
    dinv = (1.0 / np.sqrt(deg)).astype(np.float32)
    bpc = (npc + dblk - 1) // dblk

    # Balanced dst->(core, block) assignment: the per-(core, block) edge
    # counts set the chunk padding (counts are maxed over cores for the
    # shared program), so assign dsts greedily by descending edge count to
    # the least-loaded cell with free slots. The host unpermutes the output.
    import heapq
    lastcap = npc - (bpc - 1) * dblk
    cap = np.full(ncores * bpc, dblk, np.int64)
    cap[bpc - 1:: bpc] = lastcap
    slots_left = cap.copy()
    heap = [(0, c) for c in range(ncores * bpc)]
    heapq.heapify(heap)
    cell_of = np.empty(n_nodes, np.int64)
    for d_ in np.argsort(-ecnt, kind="stable"):
        while True:
            w, c = heapq.heappop(heap)
            if slots_left[c] > 0:
                break
        cell_of[d_] = c
        slots_left[c] -= 1
        heapq.heappush(heap, (w + int(ecnt[d_]), c))
    # position within cell: assignment order; newpos = core*npc + blk*128 + p
    porder = np.argsort(cell_of, kind="stable")
    newpos = np.empty(n_nodes, np.int64)
    cs = cell_of[porder]
    first = np.zeros(ncores * bpc, np.int64)
    np.add.at(first, cs, 1)
    starts = np.zeros(ncores * bpc + 1, np.int64)
    starts[1:] = np.cumsum(first)
    rankc = np.arange(n_nodes) - starts[cs]
    ck_, bk_ = cs // bpc, cs % bpc
    newpos[porder] = ck_ * npc + bk_ * dblk + rankc
    # owned[k]: old dst ids at each new position of core k
    inv = np.empty(n_nodes, np.int64)
    inv[newpos] = np.arange(n_nodes)
    owned = [inv[k * npc: (k + 1) * npc] for k in range(ncores)]

    np_dst = newpos[dst]
    core = np_dst // npc
    dloc = np_dst - core * npc
    blk = dloc // dblk

    key = core * bpc + blk
    counts = np.bincount(key, minlength=ncores * bpc).reshape(ncores, bpc)
    cmax = counts.max(axis=0)
    kblk = [max(1, -(-int(c) // P)) if c > 0 else 0 for c in cmax]
    chunk_off = np.zeros(bpc + 1, np.int64)
    chunk_off[1:] = np.cumsum(kblk)
    tot = int(chunk_off[-1])

    # group consecutive blocks until the target chunk count per dma_gather;
    # the first few groups are small so the pipeline fills quickly
    targets = [4, 7, 10, 14]
    groups = []
    cur, csum = [], 0
    rem = tot
    for b in range(bpc):
        cur.append(b)
        csum += kblk[b]
        rem -= kblk[b]
        tgt = targets[len(groups)] if len(groups) < len(targets) else GRP
        if rem < 20:
            tgt = 7
        if csum >= tgt:
            groups.append(cur)
            cur, csum = [], 0
    if cur:
        groups.append(cur)

    order = np.argsort(key, kind="stable")
    key_sorted = key[order]
    grp_start = np.zeros(ncores * bpc + 1, np.int64)
    grp_start[1:] = np.cumsum(counts.ravel())
    rank = np.arange(len(key_sorted)) - grp_start[key_sorted]

    ob, oc = blk[order], core[order]
    ck = chunk_off[ob] + rank // P
    pp = rank % P

    dstl = np.full((ncores, P, tot), -1.0, np.float32)
    dstl[oc, pp, ck] = (dloc[order] - ob * dblk).astype(np.float32)

    # per-core renumbered table ids
    uniq = []
    tid = np.zeros(len(src), np.int64)
    for k in range(ncores):
        m = core == k
        u, uinv = np.unique(src[m], return_inverse=True)
        assert len(u) <= TAB - 1, f"core {k}: {len(u)} unique srcs > {TAB-1}"
        uniq.append(u)
        tid[m] = uinv
    ot = tid[order]

    col = 8 * ck + pp // 16
    row = pp % 16
    idx16 = np.zeros((ncores, 16, 8 * tot), np.int16)
    idx16[oc, row, col] = ot.astype(np.int16)
    idx16 = np.tile(idx16, (1, 8, 1))
    return kblk, groups, idx16, dstl, uniq, dinv, owned


def _build_program(kblk, groups, alpha, n_nodes=N_NODES, ncores=NCORES,
                   in_ch=IN_CH, hid=HID):
    """Build the per-core Bass program (identical across cores)."""
    dblk = P
    npc = n_nodes // ncores
    bpc = len(kblk)
    tot = sum(kblk)
    nch = in_ch // P
    npc_pad = bpc * dblk
    gmax = max(kblk)
    chunk_off = np.zeros(bpc + 1, np.int64)
    chunk_off[1:] = np.cumsum(kblk)

    nc = bacc.Bacc(
        "TRN2", target_bir_lowering=False, debug=False,
        num_swdge_queues=4, dynamic_dma_scratch_size=32768,
    )
    x_d = nc.dram_tensor("xtab", [TAB, in_ch], BF16, kind="ExternalInput")
    si_d = nc.dram_tensor("idx16", [P, 8 * tot], mybir.dt.int16, kind="ExternalInput")
    dl_d = nc.dram_tensor("dstl", [P, tot], BF16, kind="ExternalInput")
    io_d = nc.dram_tensor("iota", [P, gmax * dblk], BF16, kind="ExternalInput")
    # xself pre-arranged on host: [p, b*in_ch+c] = xp[b*128 + p, c]
    xs_d = nc.dram_tensor("xself", [P, bpc * in_ch], BF16, kind="ExternalInput")
    dn_d = nc.dram_tensor("dnv", [P, bpc], F32, kind="ExternalInput")
    wt_ds = [
        nc.dram_tensor(f"wt{h}", [P, hid], BF16, kind="ExternalInput")
        for h in range(nch)
    ]
    bs_d = nc.dram_tensor("bias", [1, hid], BF16, kind="ExternalInput")
    on_d = nc.dram_tensor("ones", [1, P], BF16, kind="ExternalInput")
    idr_d = nc.dram_tensor("idr", [P, P], BF16, kind="ExternalInput")
    # output in block-major layout: [p, b*hid+j] = out[b*128 + p, j]
    out_d = nc.dram_tensor("out", [P, bpc * hid], BF16, kind="ExternalOutput")

    with tile.TileContext(nc) as tc, ExitStack() as ctx:
        const = ctx.enter_context(tc.tile_pool(name="const", bufs=1))
        gxp = ctx.enter_context(tc.tile_pool(name="gx", bufs=6))
        mselp = ctx.enter_context(tc.tile_pool(name="msel", bufs=4))
        psA = ctx.enter_context(tc.tile_pool(name="psA", bufs=2, space="PSUM"))
        psT = ctx.enter_context(tc.tile_pool(name="psT", bufs=2, space="PSUM"))
        hps = ctx.enter_context(tc.tile_pool(name="hps", bufs=2, space="PSUM"))
        aS = ctx.enter_context(tc.tile_pool(name="aS", bufs=3))
        outp = ctx.enter_context(tc.tile_pool(name="outp", bufs=4))
        obat = 4  # blocks per batched output write

        si_t = const.tile([P, 8 * tot], mybir.dt.int16)
        head = 8 * sum(kblk[b] for b in groups[0])
        nc.sync.dma_start(out=si_t[:, :head], in_=si_d.ap()[:, :head])
        nc.sync.dma_start(out=si_t[:, head:], in_=si_d.ap()[:, head:])
        dl_t = const.tile([P, tot], BF16)
        nc.sync.dma_start(out=dl_t[:], in_=dl_d.ap())
        io_t = const.tile([P, gmax * dblk], BF16)
        nc.sync.dma_start(out=io_t[:], in_=io_d.ap())
        dn_t = const.tile([P, bpc], F32)
        nc.sync.dma_start(out=dn_t[:], in_=dn_d.ap())
        wt_t = []
        for h in range(nch):
            w = const.tile([P, hid], BF16, name=f"wt_t{h}")
            nc.sync.dma_start(out=w[:], in_=wt_ds[h].ap())
            wt_t.append(w)
        bs_t = const.tile([1, hid], BF16)
        nc.sync.dma_start(out=bs_t[:], in_=bs_d.ap())
        on_t = const.tile([1, P], BF16)
        nc.sync.dma_start(out=on_t[:], in_=on_d.ap())
        idr_t = const.tile([P, P], BF16)
        nc.sync.dma_start(out=idr_t[:], in_=idr_d.ap())
        xs_t = const.tile([P, bpc * in_ch], BF16)
        nc.sync.dma_start(out=xs_t[:], in_=xs_d.ap())

        gather_qn = 0
        for blocks in groups:
            b0 = blocks[0]
            kg = sum(kblk[b] for b in blocks)
            gstart = int(chunk_off[b0])
            gx = None
            if kg > 0:
                nidx = kg * P
                qn = (gather_qn // 2) % NQ if os.environ.get("GCN_QPAIR", "0") == "1" else gather_qn % NQ
                gx = gxp.tile([P, kg * in_ch], BF16, tag="gx", name=f"gx_{b0}")
                nc.gpsimd.dma_gather(
                    gx[:].rearrange("p (k d) -> p k d", d=in_ch),
                    x_d.ap(),
                    si_t[:, 8 * gstart: 8 * (gstart + kg)],
                    nidx,
                    nidx,
                    in_ch,
                    queue_num=qn,
                    single_packet=False,
                )
                gather_qn += 1
            for b in blocks:
                nb = min(dblk, npc - b * dblk)
                kb = kblk[b]
                cbase = int(chunk_off[b])
                koff = cbase - gstart
                A = psA.tile([P, in_ch], F32, tag="A", name=f"A_{b}")
                first = True
                if kb > 0:
                    # one batched DVE op builds all kb selection matrices:
                    # ms[e, g, d] = (iota[e, g*128+d] == dstl[e, cbase+g])
                    ms = mselp.tile([P, kb * dblk], BF16, tag="ms", name=f"ms_{b}")
                    nc.vector.tensor_tensor(
                        out=ms[:].rearrange("p (k d) -> p k d", d=dblk),
                        in0=io_t[:, : kb * dblk].rearrange(
                            "p (k d) -> p k d", d=dblk),
                        in1=dl_t[:, cbase: cbase + kb].to_broadcast(
                            [P, kb, dblk]),
                        op=mybir.AluOpType.is_equal,
                    )
                    for j in range(kb):
                        nc.tensor.matmul(
                            A[:],
                            lhsT=ms[:, j * dblk: (j + 1) * dblk],
                            rhs=gx[:, (koff + j) * in_ch: (koff + j + 1) * in_ch],
                            start=first,
                            stop=False,
                        )
                        first = False
                # self term: A[d, :] += xp[d, :] via identity matmul
                nc.tensor.matmul(
                    A[:], lhsT=idr_t[:],
                    rhs=xs_t[:, b * in_ch: (b + 1) * in_ch],
                    start=first, stop=True,
                )
                # a_s = dinv[d] * A  (bf16), then PE-transpose halves
                a_s = aS.tile([P, in_ch], BF16, tag="as", name=f"as_{b}")
                nc.scalar.activation(
                    out=a_s[:],
                    in_=A[:],
                    func=mybir.ActivationFunctionType.Copy,
                    scale=dn_t[:, b: b + 1],
                )
                at_s = []
                for h in range(nch):
                    atp = psT.tile([P, P], BF16, tag=f"atp{h}", name=f"atp{h}_{b}")
                    nc.tensor.transpose(
                        out=atp[:], in_=a_s[:, h * P: (h + 1) * P],
                        identity=idr_t[:],
                    )
                    ats = aS.tile([P, P], BF16, tag=f"ats{h}", name=f"ats{h}_{b}")
                    nc.scalar.copy(ats[:], atp[:])
                    at_s.append(ats)
                ns = nb
                Hp = hps.tile([P, hid], F32, tag="hp", name=f"hp_{b}")
                for h in range(nch):
                    nc.tensor.matmul(
                        Hp[:ns],
                        lhsT=at_s[h][:, :ns],
                        rhs=wt_t[h][:],
                        start=(h == 0),
                        stop=False,
                    )
                nc.tensor.matmul(
                    Hp[:ns],
                    lhsT=on_t[:, :ns],
                    rhs=bs_t[:],
                    start=False,
                    stop=True,
                )
                slot = b % obat
                if slot == 0:
                    os_ = outp.tile([P, obat * hid], BF16, tag="os",
                                    name=f"os_{b}")
                osl = os_[:, slot * hid: (slot + 1) * hid]
                t2 = outp.tile([P, hid], F32, tag="t2", name=f"t2_{b}")
                if os.environ.get("GCN_PRELU", "max") == "act":
                    # single fused PReLU on the scalar engine
                    nc.scalar.activation(
                        out=osl[:ns],
                        in_=Hp[:ns],
                        func=mybir.ActivationFunctionType.Prelu,
                        scale=float(alpha),
                    )
                elif 0.0 <= alpha <= 1.0:
                    # PReLU = max(H, alpha*H)
                    nc.scalar.activation(
                        out=t2[:ns],
                        in_=Hp[:ns],
                        func=mybir.ActivationFunctionType.Copy,
                        scale=float(alpha),
                    )
                    nc.vector.tensor_tensor(
                        out=osl[:ns], in0=t2[:ns], in1=Hp[:ns],
                        op=mybir.AluOpType.max,
                    )
                else:
                    # general PReLU: relu(H)*(1-alpha) + alpha*H
                    nc.scalar.activation(
                        out=t2[:ns],
                        in_=Hp[:ns],
                        func=mybir.ActivationFunctionType.Relu,
                    )
                    nc.vector.tensor_scalar(
                        out=t2[:ns], in0=t2[:ns],
                        scalar1=float(1.0 - alpha), scalar2=None,
                        op0=mybir.AluOpType.mult,
                    )
                    t3 = outp.tile([P, hid], F32, tag="t3", name=f"t3_{b}")
                    nc.vector.tensor_scalar(
                        out=t3[:ns], in0=Hp[:ns],
                        scalar1=float(alpha), scalar2=None,
                        op0=mybir.AluOpType.mult,
                    )
                    nc.vector.tensor_tensor(
                        out=osl[:ns], in0=t2[:ns], in1=t3[:ns],
                        op=mybir.AluOpType.add,
                    )
                if slot == obat - 1 or b == bpc - 1:
                    b0 = b - slot
                    nc.sync.dma_start(
                        out=out_d.ap()[:, b0 * hid: (b + 1) * hid],
                        in_=os_[:, : (slot + 1) * hid],
                    )
    nc.compile()
    return nc


def _make_in_maps(x, weight, bias, idx16, dstl, uniq, dinv, kblk, owned,
                  ncores=NCORES):
    import ml_dtypes
    bf = ml_dtypes.bfloat16
    x = np.asarray(x, dtype=np.float32)
    w = np.asarray(weight, dtype=np.float32)
    n = x.shape[0]
    in_ch = x.shape[1]
    hid = w.shape[0]
    npc = n // ncores
    bpc = len(kblk)
    npc_pad = bpc * P
    gmax = max(kblk)
    tot = sum(kblk)

    xp = x * dinv[:, None]  # dinv[src] folded into features
    iota = np.tile(np.arange(P, dtype=np.float32), (P, gmax)).astype(bf)
    wts = {
        f"wt{h}": np.ascontiguousarray(
            w[:, h * P: (h + 1) * P].T.astype(bf))
        for h in range(in_ch // P)
    }
    bias_row = np.asarray(bias, dtype=np.float32).reshape(1, hid).astype(bf)
    in_maps = []
    for k in range(ncores):
        xtab = np.zeros((TAB, in_ch), bf)
        xtab[: len(uniq[k])] = xp[uniq[k]].astype(bf)
        xsp_ = np.zeros((npc_pad, in_ch), np.float32)
        xsp_[:npc] = xp[owned[k]]
        # [p, b*in_ch + c] = xp[b*128 + p, c]
        xs = np.ascontiguousarray(
            xsp_.reshape(bpc, P, in_ch).transpose(1, 0, 2)
            .reshape(P, bpc * in_ch)).astype(bf)
        dnv = np.zeros((P, bpc), np.float32)
        dcore = np.zeros(npc_pad, np.float32)
        dcore[:npc] = dinv[owned[k]]
        dnv[:, :] = dcore.reshape(bpc, P).T
        m = {
            "xtab": xtab,
            "idx16": np.ascontiguousarray(idx16[k]),
            "dstl": np.ascontiguousarray(dstl[k].astype(bf)),
            "iota": iota,
            "xself": xs,
            "dnv": dnv,
            "bias": bias_row,
            "ones": np.ones((1, P), bf),
            "idr": np.eye(P, dtype=bf),
        }
        m.update(wts)
        in_maps.append(m)
    return in_maps


# Results of the last kernel() call, for the test harness.
LAST_RESULTS = None


def unpack_out(arr, npc=NPC, hid=HID):
    """[128, bpc*hid] block-major device output -> [npc, hid] f32."""
    arr = np.asarray(arr)
    bpc = arr.shape[1] // hid
    full = arr.reshape(P, bpc, hid).transpose(1, 0, 2).reshape(bpc * P, hid)
    return full[:npc].astype(np.float32)


def kernel(x, edge_index, weight, bias, prelu_a):
    global LAST_RESULTS
    trace = os.environ.get("GCN_TRACE", "0") == "1"

    kblk, groups, idx16, dstl, uniq, dinv, owned = _preprocess(edge_index)
    alpha = float(np.asarray(prelu_a).ravel()[0])
    nc = _build_program(kblk, groups, alpha)
    in_maps = _make_in_maps(x, weight, bias, idx16, dstl, uniq, dinv, kblk,
                            owned)

    res = bass_utils.run_bass_kernel_spmd(
        nc, in_maps, core_ids=list(range(NCORES)), trace=trace
    )
    LAST_RESULTS = res
    dev = np.concatenate(
        [unpack_out(res.results[k]["out"]) for k in range(NCORES)], axis=0)
    out = np.empty_like(dev)
    out[np.concatenate(owned)] = dev
    return out


# revision 37
# speedup vs baseline: 1.1424x; 1.1424x over previous
"""GCN encoder (GCNConv + PReLU) as a Bass/Tile kernel on 8 Trainium2 NeuronCores.

Math (PyG GCNConv with self-loops + symmetric norm, then PReLU):
    deg[i] = in-degree over dst (+1 self loop); dinv = 1/sqrt(deg)
    agg[d] = dinv[d] * ( sum_{e:(s->d)} dinv[s]*x[s] + dinv[d]*x[d] )
    out    = PReLU(agg @ W.T + bias)

Distribution: dst-node sharding with a balanced permutation — dsts are
assigned to (core, block) cells greedily by edge count so every block needs
exactly ceil(~1020/128)=8 chunks on every core (0.35% padding); the host
unpermutes the output.

Key structure:
  - dinv[src] is folded into the features on the host (xp = dinv * x, bf16);
    dinv[dst] is applied on-chip as a per-partition scale during the
    PSUM->SBUF copy of the aggregate. Edge messages therefore need NO
    per-edge weight: the selection matrices are pure 0/1.
  - each core gathers from its OWN renumbered table of unique src rows
    (~31.6k < 32767, so a single int16-indexed dma_gather table, no halves).
    Edges are grouped by dst-block (128 dsts) and packed into 128-edge
    chunks; chunk counts are maxed over cores so all cores share a program.
  - gathers are merged: consecutive blocks are grouped until ~GRP chunks per
    dma_gather, rotated over 4 SWDGE queues.
  - per dst-block, ONE batched DVE op builds all selection matrices:
    ms[e, g*128+d] = (iota[d] == dstl[e, g]) via a stride-0 broadcast AP,
    all bf16. One PE matmul per chunk accumulates A[d,c] += ms_g^T @ gx_g.
  - the self-loop term adds as a dense identity matmul of xp rows.
  - A (f32 PSUM) is copied to bf16 with scale dinv[d], transposed on the PE
    (bf16 identity), then H = A^T W^T + bias accumulates in PSUM (bf16 in,
    f32 accum). PReLU = max(H, alpha*H) for 0<=alpha<=1, general fallback
    relu(H)*(1-alpha) + alpha*H.
"""

import os
import numpy as np
from contextlib import ExitStack

import concourse.tile as tile
from concourse import bacc, mybir, bass_utils

# Problem shape (fixed by the harness contract).
N_NODES = 50000
N_EDGES = 400000
IN_CH = 256
HID = 512
NCORES = 8
NPC = N_NODES // NCORES  # dst nodes owned per core
P = 128
TAB = 32768              # gather table rows (unique srcs per core, padded)

F32 = mybir.dt.float32
BF16 = mybir.dt.bfloat16
# target chunks per merged dma_gather instruction
GRP = int(os.environ.get("GCN_GRP", "18"))
# number of SWDGE queues to rotate gathers over
NQ = int(os.environ.get("GCN_NQ", "4"))


def _preprocess(edge_index, n_nodes=N_NODES, ncores=NCORES):
    """Per-core edge packing with renumbered unique-src gather tables.

    Returns (kblk, groups, idx16, dstl, uniq, dinv):
      kblk:   [bpc] chunks per dst-block (compile-time, maxed over cores)
      groups: list of lists of consecutive block ids, ~GRP chunks per group
      idx16:  [ncores, 128, 8*tot] int16 gather indices (16-wrap, 8x tiled)
      dstl:   [ncores, 128, tot] f32 dst-local-in-block per edge slot (-1 pad)
      uniq:   per-core sorted unique src ids (table content order)
      dinv:   [n_nodes] f32 1/sqrt(deg)
    """
    dblk = P
    npc = n_nodes // ncores
    src = np.asarray(edge_index[0]).astype(np.int64).ravel()
    dst = np.asarray(edge_index[1]).astype(np.int64).ravel()
    ecnt = np.bincount(dst, minlength=n_nodes)
    deg = ecnt.astype(np.float32) + 1.0
    dinv = (1.0 / np.sqrt(deg)).astype(np.float32)
    bpc = (npc + dblk - 1) // dblk

    # Balanced dst->(core, block) assignment: the per-(core, block) edge
    # counts set the chunk padding (counts are maxed over cores for the
    # shared program), so assign dsts greedily by descending edge count to
    # the least-loaded cell with free slots. The host unpermutes the output.
    import heapq
    lastcap = npc - (bpc - 1) * dblk
    cap = np.full(ncores * bpc, dblk, np.int64)
    cap[bpc - 1:: bpc] = lastcap
    slots_left = cap.copy()
    heap = [(0, c) for c in range(ncores * bpc)]
    heapq.heapify(heap)
    cell_of = np.empty(n_nodes, np.int64)
    for d_ in np.argsort(-ecnt, kind="stable"):
        while True:
            w, c = heapq.heappop(heap)
            if slots_left[c] > 0:
                break
        cell_of[d_] = c
        slots_left[c] -= 1
        heapq.heappush(heap, (w + int(ecnt[d_]), c))
    # position within cell: assignment order; newpos = core*npc + blk*128 + p
    porder = np.argsort(cell_of, kind="stable")
    newpos = np.empty(n_nodes, np.int64)
    cs = cell_of[porder]
    first = np.zeros(ncores * bpc, np.int64)
    np.add.at(first, cs, 1)
    starts = np.zeros(ncores * bpc + 1, np.int64)
    starts[1:] = np.cumsum(first)
    rankc = np.arange(n_nodes) - starts[cs]
    ck_, bk_ = cs // bpc, cs % bpc
    newpos[porder] = ck_ * npc + bk_ * dblk + rankc
    # owned[k]: old dst ids at each new position of core k
    inv = np.empty(n_nodes, np.int64)
    inv[newpos] = np.arange(n_nodes)
    owned = [inv[k * npc: (k + 1) * npc] for k in range(ncores)]

    np_dst = newpos[dst]
    core = np_dst // npc
    dloc = np_dst - core * npc
    blk = dloc // dblk

    key = core * bpc + blk
    counts = np.bincount(key, minlength=ncores * bpc).reshape(ncores, bpc)
    cmax = counts.max(axis=0)
    kblk = [max(1, -(-int(c) // P)) if c > 0 else 0 for c in cmax]
    chunk_off = np.zeros(bpc + 1, np.int64)
    chunk_off[1:] = np.cumsum(kblk)
    tot = int(chunk_off[-1])

    # group consecutive blocks until the target chunk count per dma_gather;
    # the first few groups are small so the pipeline fills quickly
    targets = [4, 7, 10, 14]
    groups = []
    cur, csum = [], 0
    rem = tot
    for b in range(bpc):
        cur.append(b)
        csum += kblk[b]
        rem -= kblk[b]
        tgt = targets[len(groups)] if len(groups) < len(targets) else GRP
        if rem < 2 * GRP:
            tgt = 6
        if csum >= tgt:
            groups.append(cur)
            cur, csum = [], 0
    if cur:
        groups.append(cur)

    order = np.argsort(key, kind="stable")
    key_sorted = key[order]
    grp_start = np.zeros(ncores * bpc + 1, np.int64)
    grp_start[1:] = np.cumsum(counts.ravel())
    rank = np.arange(len(key_sorted)) - grp_start[key_sorted]

    ob, oc = blk[order], core[order]
    ck = chunk_off[ob] + rank // P
    pp = rank % P

    dstl = np.full((ncores, P, tot), -1.0, np.float32)
    dstl[oc, pp, ck] = (dloc[order] - ob * dblk).astype(np.float32)

    # per-core renumbered table ids
    uniq = []
    tid = np.zeros(len(src), np.int64)
    for k in range(ncores):
        m = core == k
        u, uinv = np.unique(src[m], return_inverse=True)
        assert len(u) <= TAB - 1, f"core {k}: {len(u)} unique srcs > {TAB-1}"
        uniq.append(u)
        tid[m] = uinv
    ot = tid[order]

    col = 8 * ck + pp // 16
    row = pp % 16
    idx16 = np.zeros((ncores, 16, 8 * tot), np.int16)
    idx16[oc, row, col] = ot.astype(np.int16)
    idx16 = np.tile(idx16, (1, 8, 1))
    return kblk, groups, idx16, dstl, uniq, dinv, owned


def _build_program(kblk, groups, alpha, n_nodes=N_NODES, ncores=NCORES,
                   in_ch=IN_CH, hid=HID):
    """Build the per-core Bass program (identical across cores)."""
    dblk = P
    npc = n_nodes // ncores
    bpc = len(kblk)
    tot = sum(kblk)
    nch = in_ch // P
    npc_pad = bpc * dblk
    gmax = max(kblk)
    chunk_off = np.zeros(bpc + 1, np.int64)
    chunk_off[1:] = np.cumsum(kblk)

    nc = bacc.Bacc(
        "TRN2", target_bir_lowering=False, debug=False,
        num_swdge_queues=4, dynamic_dma_scratch_size=32768,
    )
    x_d = nc.dram_tensor("xtab", [TAB, in_ch], BF16, kind="ExternalInput")
    si_d = nc.dram_tensor("idx16", [P, 8 * tot], mybir.dt.int16, kind="ExternalInput")
    dl_d = nc.dram_tensor("dstl", [P, tot], BF16, kind="ExternalInput")
    io_d = nc.dram_tensor("iota", [P, gmax * dblk], BF16, kind="ExternalInput")
    # xself pre-arranged on host: [p, b*in_ch+c] = xp[b*128 + p, c]
    xs_d = nc.dram_tensor("xself", [P, bpc * in_ch], BF16, kind="ExternalInput")
    dn_d = nc.dram_tensor("dnv", [P, bpc], F32, kind="ExternalInput")
    wt_ds = [
        nc.dram_tensor(f"wt{h}", [P, hid], BF16, kind="ExternalInput")
        for h in range(nch)
    ]
    bs_d = nc.dram_tensor("bias", [1, hid], BF16, kind="ExternalInput")
    on_d = nc.dram_tensor("ones", [1, P], BF16, kind="ExternalInput")
    idr_d = nc.dram_tensor("idr", [P, P], BF16, kind="ExternalInput")
    # output in block-major layout: [p, b*hid+j] = out[b*128 + p, j]
    out_d = nc.dram_tensor("out", [P, bpc * hid], BF16, kind="ExternalOutput")

    with tile.TileContext(nc) as tc, ExitStack() as ctx:
        const = ctx.enter_context(tc.tile_pool(name="const", bufs=1))
        gxp = ctx.enter_context(tc.tile_pool(name="gx", bufs=6))
        mselp = ctx.enter_context(tc.tile_pool(name="msel", bufs=4))
        psA = ctx.enter_context(tc.tile_pool(name="psA", bufs=2, space="PSUM"))
        psT = ctx.enter_context(tc.tile_pool(name="psT", bufs=2, space="PSUM"))
        hps = ctx.enter_context(tc.tile_pool(name="hps", bufs=2, space="PSUM"))
        aS = ctx.enter_context(tc.tile_pool(name="aS", bufs=3))
        outp = ctx.enter_context(tc.tile_pool(name="outp", bufs=4))
        obat = 4  # blocks per batched output write

        si_t = const.tile([P, 8 * tot], mybir.dt.int16)
        head = 8 * sum(kblk[b] for b in groups[0])
        nc.sync.dma_start(out=si_t[:, :head], in_=si_d.ap()[:, :head])
        nc.sync.dma_start(out=si_t[:, head:], in_=si_d.ap()[:, head:])
        dl_t = const.tile([P, tot], BF16)
        nc.sync.dma_start(out=dl_t[:], in_=dl_d.ap())
        io_t = const.tile([P, gmax * dblk], BF16)
        nc.sync.dma_start(out=io_t[:], in_=io_d.ap())
        dn_t = const.tile([P, bpc], F32)
        nc.sync.dma_start(out=dn_t[:], in_=dn_d.ap())
        wt_t = []
        for h in range(nch):
            w = const.tile([P, hid], BF16, name=f"wt_t{h}")
            nc.sync.dma_start(out=w[:], in_=wt_ds[h].ap())
            wt_t.append(w)
        bs_t = const.tile([1, hid], BF16)
        nc.sync.dma_start(out=bs_t[:], in_=bs_d.ap())
        on_t = const.tile([1, P], BF16)
        nc.sync.dma_start(out=on_t[:], in_=on_d.ap())
        idr_t = const.tile([P, P], BF16)
        nc.sync.dma_start(out=idr_t[:], in_=idr_d.ap())
        xs_t = const.tile([P, bpc * in_ch], BF16)
        nc.sync.dma_start(out=xs_t[:], in_=xs_d.ap())

        gather_qn = 0
        for blocks in groups:
            b0 = blocks[0]
            kg = sum(kblk[b] for b in blocks)
            gstart = int(chunk_off[b0])
            gx = None
            if kg > 0:
                nidx = kg * P
                qn = (gather_qn // 2) % NQ if os.environ.get("GCN_QPAIR", "0") == "1" else gather_qn % NQ
                gx = gxp.tile([P, kg * in_ch], BF16, tag="gx", name=f"gx_{b0}")
                nc.gpsimd.dma_gather(
                    gx[:].rearrange("p (k d) -> p k d", d=in_ch),
                    x_d.ap(),
                    si_t[:, 8 * gstart: 8 * (gstart + kg)],
                    nidx,
                    nidx,
                    in_ch,
                    queue_num=qn,
                    single_packet=False,
                )
                gather_qn += 1
            for b in blocks:
                nb = min(dblk, npc - b * dblk)
                kb = kblk[b]
                cbase = int(chunk_off[b])
                koff = cbase - gstart
                A = psA.tile([P, in_ch], F32, tag="A", name=f"A_{b}")
                first = True
                if kb > 0:
                    # one batched DVE op builds all kb selection matrices:
                    # ms[e, g, d] = (iota[e, g*128+d] == dstl[e, cbase+g])
                    ms = mselp.tile([P, kb * dblk], BF16, tag="ms", name=f"ms_{b}")
                    nc.vector.tensor_tensor(
                        out=ms[:].rearrange("p (k d) -> p k d", d=dblk),
                        in0=io_t[:, : kb * dblk].rearrange(
                            "p (k d) -> p k d", d=dblk),
                        in1=dl_t[:, cbase: cbase + kb].to_broadcast(
                            [P, kb, dblk]),
                        op=mybir.AluOpType.is_equal,
                    )
                    for j in range(kb):
                        nc.tensor.matmul(
                            A[:],
                            lhsT=ms[:, j * dblk: (j + 1) * dblk],
                            rhs=gx[:, (koff + j) * in_ch: (koff + j + 1) * in_ch],
                            start=first,
                            stop=False,
                        )
                        first = False
                # self term: A[d, :] += xp[d, :] via identity matmul
                nc.tensor.matmul(
                    A[:], lhsT=idr_t[:],
                    rhs=xs_t[:, b * in_ch: (b + 1) * in_ch],
                    start=first, stop=True,
                )
                # a_s = dinv[d] * A  (bf16), then PE-transpose halves
                a_s = aS.tile([P, in_ch], BF16, tag="as", name=f"as_{b}")
                nc.scalar.activation(
                    out=a_s[:],
                    in_=A[:],
                    func=mybir.ActivationFunctionType.Copy,
                    scale=dn_t[:, b: b + 1],
                )
                at_s = []
                for h in range(nch):
                    atp = psT.tile([P, P], BF16, tag=f"atp{h}", name=f"atp{h}_{b}")
                    nc.tensor.transpose(
                        out=atp[:], in_=a_s[:, h * P: (h + 1) * P],
                        identity=idr_t[:],
                    )
                    ats = aS.tile([P, P], BF16, tag=f"ats{h}", name=f"ats{h}_{b}")
                    nc.scalar.copy(ats[:], atp[:])
                    at_s.append(ats)
                ns = nb
                Hp = hps.tile([P, hid], F32, tag="hp", name=f"hp_{b}")
                for h in range(nch):
                    nc.tensor.matmul(
                        Hp[:ns],
                        lhsT=at_s[h][:, :ns],
                        rhs=wt_t[h][:],
                        start=(h == 0),
                        stop=False,
                    )
                nc.tensor.matmul(
                    Hp[:ns],
                    lhsT=on_t[:, :ns],
                    rhs=bs_t[:],
                    start=False,
                    stop=True,
                )
                slot = b % obat
                if slot == 0:
                    os_ = outp.tile([P, obat * hid], BF16, tag="os",
                                    name=f"os_{b}")
                osl = os_[:, slot * hid: (slot + 1) * hid]
                t2 = outp.tile([P, hid], F32, tag="t2", name=f"t2_{b}")
                if os.environ.get("GCN_PRELU", "max") == "act":
                    # single fused PReLU on the scalar engine
                    nc.scalar.activation(
                        out=osl[:ns],
                        in_=Hp[:ns],
                        func=mybir.ActivationFunctionType.Prelu,
                        scale=float(alpha),
                    )
                elif 0.0 <= alpha <= 1.0:
                    # PReLU = max(H, alpha*H)
                    nc.scalar.activation(
                        out=t2[:ns],
                        in_=Hp[:ns],
                        func=mybir.ActivationFunctionType.Copy,
                        scale=float(alpha),
                    )
                    nc.vector.tensor_tensor(
                        out=osl[:ns], in0=t2[:ns], in1=Hp[:ns],
                        op=mybir.AluOpType.max,
                    )
                else:
                    # general PReLU: relu(H)*(1-alpha) + alpha*H
                    nc.scalar.activation(
                        out=t2[:ns],
                        in_=Hp[:ns],
                        func=mybir.ActivationFunctionType.Relu,
                    )
                    nc.vector.tensor_scalar(
                        out=t2[:ns], in0=t2[:ns],
                        scalar1=float(1.0 - alpha), scalar2=None,
                        op0=mybir.AluOpType.mult,
                    )
                    t3 = outp.tile([P, hid], F32, tag="t3", name=f"t3_{b}")
                    nc.vector.tensor_scalar(
                        out=t3[:ns], in0=Hp[:ns],
                        scalar1=float(alpha), scalar2=None,
                        op0=mybir.AluOpType.mult,
                    )
                    nc.vector.tensor_tensor(
                        out=osl[:ns], in0=t2[:ns], in1=t3[:ns],
                        op=mybir.AluOpType.add,
                    )
                if slot == obat - 1 or b == bpc - 1:
                    b0 = b - slot
                    nc.sync.dma_start(
                        out=out_d.ap()[:, b0 * hid: (b + 1) * hid],
                        in_=os_[:, : (slot + 1) * hid],
                    )
    nc.compile()
    return nc


def _make_in_maps(x, weight, bias, idx16, dstl, uniq, dinv, kblk, owned,
                  ncores=NCORES):
    import ml_dtypes
    bf = ml_dtypes.bfloat16
    x = np.asarray(x, dtype=np.float32)
    w = np.asarray(weight, dtype=np.float32)
    n = x.shape[0]
    in_ch = x.shape[1]
    hid = w.shape[0]
    npc = n // ncores
    bpc = len(kblk)
    npc_pad = bpc * P
    gmax = max(kblk)
    tot = sum(kblk)

    xp = x * dinv[:, None]  # dinv[src] folded into features
    iota = np.tile(np.arange(P, dtype=np.float32), (P, gmax)).astype(bf)
    wts = {
        f"wt{h}": np.ascontiguousarray(
            w[:, h * P: (h + 1) * P].T.astype(bf))
        for h in range(in_ch // P)
    }
    bias_row = np.asarray(bias, dtype=np.float32).reshape(1, hid).astype(bf)
    in_maps = []
    for k in range(ncores):
        xtab = np.zeros((TAB, in_ch), bf)
        xtab[: len(uniq[k])] = xp[uniq[k]].astype(bf)
        xsp_ = np.zeros((npc_pad, in_ch), np.float32)
        xsp_[:npc] = xp[owned[k]]
        # [p, b*in_ch + c] = xp[b*128 + p, c]
        xs = np.ascontiguousarray(
            xsp_.reshape(bpc, P, in_ch).transpose(1, 0, 2)
            .reshape(P, bpc * in_ch)).astype(bf)
        dnv = np.zeros((P, bpc), np.float32)
        dcore = np.zeros(npc_pad, np.float32)
        dcore[:npc] = dinv[owned[k]]
        dnv[:, :] = dcore.reshape(bpc, P).T
        m = {
            "xtab": xtab,
            "idx16": np.ascontiguousarray(idx16[k]),
            "dstl": np.ascontiguousarray(dstl[k].astype(bf)),
            "iota": iota,
            "xself": xs,
            "dnv": dnv,
            "bias": bias_row,
            "ones": np.ones((1, P), bf),
            "idr": np.eye(P, dtype=bf),
        }
        m.update(wts)
        in_maps.append(m)
    return in_maps


# Results of the last kernel() call, for the test harness.
LAST_RESULTS = None


def unpack_out(arr, npc=NPC, hid=HID):
    """[128, bpc*hid] block-major device output -> [npc, hid] f32."""
    arr = np.asarray(arr)
    bpc = arr.shape[1] // hid
    full = arr.reshape(P, bpc, hid).transpose(1, 0, 2).reshape(bpc * P, hid)
    return full[:npc].astype(np.float32)


def kernel(x, edge_index, weight, bias, prelu_a):
    global LAST_RESULTS
    trace = os.environ.get("GCN_TRACE", "0") == "1"

    kblk, groups, idx16, dstl, uniq, dinv, owned = _preprocess(edge_index)
    alpha = float(np.asarray(prelu_a).ravel()[0])
    nc = _build_program(kblk, groups, alpha)
    in_maps = _make_in_maps(x, weight, bias, idx16, dstl, uniq, dinv, kblk,
                            owned)

    res = bass_utils.run_bass_kernel_spmd(
        nc, in_maps, core_ids=list(range(NCORES)), trace=trace
    )
    LAST_RESULTS = res
    dev = np.concatenate(
        [unpack_out(res.results[k]["out"]) for k in range(NCORES)], axis=0)
    out = np.empty_like(dev)
    out[np.concatenate(owned)] = dev
    return out


# revision 38
# speedup vs baseline: 1.1560x; 1.0119x over previous
"""GCN encoder (GCNConv + PReLU) as a Bass/Tile kernel on 8 Trainium2 NeuronCores.

Math (PyG GCNConv with self-loops + symmetric norm, then PReLU):
    deg[i] = in-degree over dst (+1 self loop); dinv = 1/sqrt(deg)
    agg[d] = dinv[d] * ( sum_{e:(s->d)} dinv[s]*x[s] + dinv[d]*x[d] )
    out    = PReLU(agg @ W.T + bias)

Distribution: dst-node sharding with a balanced permutation — dsts are
assigned to (core, block) cells greedily by edge count so every block needs
exactly ceil(~1020/128)=8 chunks on every core (0.35% padding); the host
unpermutes the output.

Key structure:
  - dinv[src] is folded into the features on the host (xp = dinv * x, bf16);
    dinv[dst] is applied on-chip as a per-partition scale during the
    PSUM->SBUF copy of the aggregate. Edge messages therefore need NO
    per-edge weight: the selection matrices are pure 0/1.
  - each core gathers from its OWN renumbered table of unique src rows
    (~31.6k < 32767, so a single int16-indexed dma_gather table, no halves).
    Edges are grouped by dst-block (128 dsts) and packed into 128-edge
    chunks; chunk counts are maxed over cores so all cores share a program.
  - gathers are merged: consecutive blocks are grouped until ~GRP chunks per
    dma_gather, rotated over 4 SWDGE queues.
  - per dst-block, ONE batched DVE op builds all selection matrices:
    ms[e, g*128+d] = (iota[d] == dstl[e, g]) via a stride-0 broadcast AP,
    all bf16. One PE matmul per chunk accumulates A[d,c] += ms_g^T @ gx_g.
  - the self-loop term adds as a dense identity matmul of xp rows.
  - A (f32 PSUM) is copied to bf16 with scale dinv[d], transposed on the PE
    (bf16 identity), then H = A^T W^T + bias accumulates in PSUM (bf16 in,
    f32 accum). PReLU = max(H, alpha*H) for 0<=alpha<=1, general fallback
    relu(H)*(1-alpha) + alpha*H.
"""

import os
import numpy as np
from contextlib import ExitStack

import concourse.tile as tile
from concourse import bacc, mybir, bass_utils

# Problem shape (fixed by the harness contract).
N_NODES = 50000
N_EDGES = 400000
IN_CH = 256
HID = 512
NCORES = 8
NPC = N_NODES // NCORES  # dst nodes owned per core
P = 128
TAB = 32768              # gather table rows (unique srcs per core, padded)

F32 = mybir.dt.float32
BF16 = mybir.dt.bfloat16
# target chunks per merged dma_gather instruction
GRP = int(os.environ.get("GCN_GRP", "18"))
# number of SWDGE queues to rotate gathers over
NQ = int(os.environ.get("GCN_NQ", "4"))


def _preprocess(edge_index, n_nodes=N_NODES, ncores=NCORES):
    """Per-core edge packing with renumbered unique-src gather tables.

    Returns (kblk, groups, idx16, dstl, uniq, dinv):
      kblk:   [bpc] chunks per dst-block (compile-time, maxed over cores)
      groups: list of lists of consecutive block ids, ~GRP chunks per group
      idx16:  [ncores, 128, 8*tot] int16 gather indices (16-wrap, 8x tiled)
      dstl:   [ncores, 128, tot] f32 dst-local-in-block per edge slot (-1 pad)
      uniq:   per-core sorted unique src ids (table content order)
      dinv:   [n_nodes] f32 1/sqrt(deg)
    """
    dblk = P
    npc = n_nodes // ncores
    src = np.asarray(edge_index[0]).astype(np.int64).ravel()
    dst = np.asarray(edge_index[1]).astype(np.int64).ravel()
    ecnt = np.bincount(dst, minlength=n_nodes)
    deg = ecnt.astype(np.float32) + 1.0
    dinv = (1.0 / np.sqrt(deg)).astype(np.float32)
    bpc = (npc + dblk - 1) // dblk

    # Balanced dst->(core, block) assignment: the per-(core, block) edge
    # counts set the chunk padding (counts are maxed over cores for the
    # shared program), so assign dsts greedily by descending edge count to
    # the least-loaded cell with free slots. The host unpermutes the output.
    import heapq
    lastcap = npc - (bpc - 1) * dblk
    cap = np.full(ncores * bpc, dblk, np.int64)
    cap[bpc - 1:: bpc] = lastcap
    slots_left = cap.copy()
    heap = [(0, c) for c in range(ncores * bpc)]
    heapq.heapify(heap)
    cell_of = np.empty(n_nodes, np.int64)
    for d_ in np.argsort(-ecnt, kind="stable"):
        while True:
            w, c = heapq.heappop(heap)
            if slots_left[c] > 0:
                break
        cell_of[d_] = c
        slots_left[c] -= 1
        heapq.heappush(heap, (w + int(ecnt[d_]), c))
    # position within cell: assignment order; newpos = core*npc + blk*128 + p
    porder = np.argsort(cell_of, kind="stable")
    newpos = np.empty(n_nodes, np.int64)
    cs = cell_of[porder]
    first = np.zeros(ncores * bpc, np.int64)
    np.add.at(first, cs, 1)
    starts = np.zeros(ncores * bpc + 1, np.int64)
    starts[1:] = np.cumsum(first)
    rankc = np.arange(n_nodes) - starts[cs]
    ck_, bk_ = cs // bpc, cs % bpc
    newpos[porder] = ck_ * npc + bk_ * dblk + rankc
    # owned[k]: old dst ids at each new position of core k
    inv = np.empty(n_nodes, np.int64)
    inv[newpos] = np.arange(n_nodes)
    owned = [inv[k * npc: (k + 1) * npc] for k in range(ncores)]

    np_dst = newpos[dst]
    core = np_dst // npc
    dloc = np_dst - core * npc
    blk = dloc // dblk

    key = core * bpc + blk
    counts = np.bincount(key, minlength=ncores * bpc).reshape(ncores, bpc)
    cmax = counts.max(axis=0)
    kblk = [max(1, -(-int(c) // P)) if c > 0 else 0 for c in cmax]
    chunk_off = np.zeros(bpc + 1, np.int64)
    chunk_off[1:] = np.cumsum(kblk)
    tot = int(chunk_off[-1])

    # group consecutive blocks until the target chunk count per dma_gather;
    # the first few groups are small so the pipeline fills quickly
    targets = [4, 7, 10, 14]
    groups = []
    cur, csum = [], 0
    rem = tot
    for b in range(bpc):
        cur.append(b)
        csum += kblk[b]
        rem -= kblk[b]
        tgt = targets[len(groups)] if len(groups) < len(targets) else GRP
        if rem < 2 * GRP:
            tgt = 6
        if csum >= tgt:
            groups.append(cur)
            cur, csum = [], 0
    if cur:
        groups.append(cur)

    order = np.argsort(key, kind="stable")
    key_sorted = key[order]
    grp_start = np.zeros(ncores * bpc + 1, np.int64)
    grp_start[1:] = np.cumsum(counts.ravel())
    rank = np.arange(len(key_sorted)) - grp_start[key_sorted]

    ob, oc = blk[order], core[order]
    ck = chunk_off[ob] + rank // P
    pp = rank % P

    dstl = np.full((ncores, P, tot), -1.0, np.float32)
    dstl[oc, pp, ck] = (dloc[order] - ob * dblk).astype(np.float32)

    # per-core renumbered table ids
    uniq = []
    tid = np.zeros(len(src), np.int64)
    for k in range(ncores):
        m = core == k
        u, uinv = np.unique(src[m], return_inverse=True)
        assert len(u) <= TAB - 1, f"core {k}: {len(u)} unique srcs > {TAB-1}"
        uniq.append(u)
        tid[m] = uinv
    ot = tid[order]

    col = 8 * ck + pp // 16
    row = pp % 16
    idx16 = np.zeros((ncores, 16, 8 * tot), np.int16)
    idx16[oc, row, col] = ot.astype(np.int16)
    idx16 = np.tile(idx16, (1, 8, 1))
    return kblk, groups, idx16, dstl, uniq, dinv, owned


def _build_program(kblk, groups, alpha, n_nodes=N_NODES, ncores=NCORES,
                   in_ch=IN_CH, hid=HID):
    """Build the per-core Bass program (identical across cores)."""
    dblk = P
    npc = n_nodes // ncores
    bpc = len(kblk)
    tot = sum(kblk)
    nch = in_ch // P
    npc_pad = bpc * dblk
    gmax = max(kblk)
    chunk_off = np.zeros(bpc + 1, np.int64)
    chunk_off[1:] = np.cumsum(kblk)

    nc = bacc.Bacc(
        "TRN2", target_bir_lowering=False, debug=False,
        num_swdge_queues=4, dynamic_dma_scratch_size=32768,
    )
    x_d = nc.dram_tensor("xtab", [TAB, in_ch], BF16, kind="ExternalInput")
    si_d = nc.dram_tensor("idx16", [P, 8 * tot], mybir.dt.int16, kind="ExternalInput")
    dl_d = nc.dram_tensor("dstl", [P, tot], BF16, kind="ExternalInput")
    io_d = nc.dram_tensor("iota", [P, gmax * dblk], BF16, kind="ExternalInput")
    # xself pre-arranged on host: [p, b*in_ch+c] = xp[b*128 + p, c]
    xs_d = nc.dram_tensor("xself", [P, bpc * in_ch], BF16, kind="ExternalInput")
    dn_d = nc.dram_tensor("dnv", [P, bpc], F32, kind="ExternalInput")
    wt_ds = [
        nc.dram_tensor(f"wt{h}", [P, hid], BF16, kind="ExternalInput")
        for h in range(nch)
    ]
    bs_d = nc.dram_tensor("bias", [1, hid], BF16, kind="ExternalInput")
    on_d = nc.dram_tensor("ones", [1, P], BF16, kind="ExternalInput")
    idr_d = nc.dram_tensor("idr", [P, P], BF16, kind="ExternalInput")
    # output in block-major layout: [p, b*hid+j] = out[b*128 + p, j]
    out_d = nc.dram_tensor("out", [P, bpc * hid], BF16, kind="ExternalOutput")

    with tile.TileContext(nc) as tc, ExitStack() as ctx:
        const = ctx.enter_context(tc.tile_pool(name="const", bufs=1))
        gxp = ctx.enter_context(tc.tile_pool(name="gx", bufs=6))
        mselp = ctx.enter_context(tc.tile_pool(name="msel", bufs=6))
        psA = ctx.enter_context(tc.tile_pool(name="psA", bufs=2, space="PSUM"))
        psT = ctx.enter_context(tc.tile_pool(name="psT", bufs=2, space="PSUM"))
        hps = ctx.enter_context(tc.tile_pool(name="hps", bufs=2, space="PSUM"))
        aS = ctx.enter_context(tc.tile_pool(name="aS", bufs=4))
        outp = ctx.enter_context(tc.tile_pool(name="outp", bufs=4))
        obat = 4  # blocks per batched output write

        si_t = const.tile([P, 8 * tot], mybir.dt.int16)
        head = 8 * sum(kblk[b] for b in groups[0])
        nc.sync.dma_start(out=si_t[:, :head], in_=si_d.ap()[:, :head])
        nc.sync.dma_start(out=si_t[:, head:], in_=si_d.ap()[:, head:])
        dl_t = const.tile([P, tot], BF16)
        nc.sync.dma_start(out=dl_t[:], in_=dl_d.ap())
        io_t = const.tile([P, gmax * dblk], BF16)
        nc.sync.dma_start(out=io_t[:], in_=io_d.ap())
        dn_t = const.tile([P, bpc], F32)
        nc.sync.dma_start(out=dn_t[:], in_=dn_d.ap())
        wt_t = []
        for h in range(nch):
            w = const.tile([P, hid], BF16, name=f"wt_t{h}")
            nc.sync.dma_start(out=w[:], in_=wt_ds[h].ap())
            wt_t.append(w)
        bs_t = const.tile([1, hid], BF16)
        nc.sync.dma_start(out=bs_t[:], in_=bs_d.ap())
        on_t = const.tile([1, P], BF16)
        nc.sync.dma_start(out=on_t[:], in_=on_d.ap())
        idr_t = const.tile([P, P], BF16)
        nc.sync.dma_start(out=idr_t[:], in_=idr_d.ap())
        xs_t = const.tile([P, bpc * in_ch], BF16)
        nc.sync.dma_start(out=xs_t[:], in_=xs_d.ap())

        gather_qn = 0
        for blocks in groups:
            b0 = blocks[0]
            kg = sum(kblk[b] for b in blocks)
            gstart = int(chunk_off[b0])
            gx = None
            if kg > 0:
                nidx = kg * P
                qn = (gather_qn // 2) % NQ if os.environ.get("GCN_QPAIR", "0") == "1" else gather_qn % NQ
                gx = gxp.tile([P, kg * in_ch], BF16, tag="gx", name=f"gx_{b0}")
                nc.gpsimd.dma_gather(
                    gx[:].rearrange("p (k d) -> p k d", d=in_ch),
                    x_d.ap(),
                    si_t[:, 8 * gstart: 8 * (gstart + kg)],
                    nidx,
                    nidx,
                    in_ch,
                    queue_num=qn,
                    single_packet=False,
                )
                gather_qn += 1
            for b in blocks:
                nb = min(dblk, npc - b * dblk)
                kb = kblk[b]
                cbase = int(chunk_off[b])
                koff = cbase - gstart
                A = psA.tile([P, in_ch], F32, tag="A", name=f"A_{b}")
                first = True
                if kb > 0:
                    # one batched DVE op builds all kb selection matrices:
                    # ms[e, g, d] = (iota[e, g*128+d] == dstl[e, cbase+g])
                    ms = mselp.tile([P, kb * dblk], BF16, tag="ms", name=f"ms_{b}")
                    nc.vector.tensor_tensor(
                        out=ms[:].rearrange("p (k d) -> p k d", d=dblk),
                        in0=io_t[:, : kb * dblk].rearrange(
                            "p (k d) -> p k d", d=dblk),
                        in1=dl_t[:, cbase: cbase + kb].to_broadcast(
                            [P, kb, dblk]),
                        op=mybir.AluOpType.is_equal,
                    )
                    for j in range(kb):
                        nc.tensor.matmul(
                            A[:],
                            lhsT=ms[:, j * dblk: (j + 1) * dblk],
                            rhs=gx[:, (koff + j) * in_ch: (koff + j + 1) * in_ch],
                            start=first,
                            stop=False,
                        )
                        first = False
                # self term: A[d, :] += xp[d, :] via identity matmul
                nc.tensor.matmul(
                    A[:], lhsT=idr_t[:],
                    rhs=xs_t[:, b * in_ch: (b + 1) * in_ch],
                    start=first, stop=True,
                )
                # a_s = dinv[d] * A  (bf16), then PE-transpose halves
                a_s = aS.tile([P, in_ch], BF16, tag="as", name=f"as_{b}")
                nc.scalar.activation(
                    out=a_s[:],
                    in_=A[:],
                    func=mybir.ActivationFunctionType.Copy,
                    scale=dn_t[:, b: b + 1],
                )
                at_s = []
                for h in range(nch):
                    atp = psT.tile([P, P], BF16, tag=f"atp{h}", name=f"atp{h}_{b}")
                    nc.tensor.transpose(
                        out=atp[:], in_=a_s[:, h * P: (h + 1) * P],
                        identity=idr_t[:],
                    )
                    ats = aS.tile([P, P], BF16, tag=f"ats{h}", name=f"ats{h}_{b}")
                    nc.scalar.copy(ats[:], atp[:])
                    at_s.append(ats)
                ns = nb
                Hp = hps.tile([P, hid], F32, tag="hp", name=f"hp_{b}")
                for h in range(nch):
                    nc.tensor.matmul(
                        Hp[:ns],
                        lhsT=at_s[h][:, :ns],
                        rhs=wt_t[h][:],
                        start=(h == 0),
                        stop=False,
                    )
                nc.tensor.matmul(
                    Hp[:ns],
                    lhsT=on_t[:, :ns],
                    rhs=bs_t[:],
                    start=False,
                    stop=True,
                )
                slot = b % obat
                if slot == 0:
                    os_ = outp.tile([P, obat * hid], BF16, tag="os",
                                    name=f"os_{b}")
                osl = os_[:, slot * hid: (slot + 1) * hid]
                t2 = outp.tile([P, hid], F32, tag="t2", name=f"t2_{b}")
                if os.environ.get("GCN_PRELU", "max") == "act":
                    # single fused PReLU on the scalar engine
                    nc.scalar.activation(
                        out=osl[:ns],
                        in_=Hp[:ns],
                        func=mybir.ActivationFunctionType.Prelu,
                        scale=float(alpha),
                    )
                elif 0.0 <= alpha <= 1.0:
                    # PReLU = max(H, alpha*H)
                    nc.scalar.activation(
                        out=t2[:ns],
                        in_=Hp[:ns],
                        func=mybir.ActivationFunctionType.Copy,
                        scale=float(alpha),
                    )
                    nc.vector.tensor_tensor(
                        out=osl[:ns], in0=t2[:ns], in1=Hp[:ns],
                        op=mybir.AluOpType.max,
                    )
                else:
                    # general PReLU: relu(H)*(1-alpha) + alpha*H
                    nc.scalar.activation(
                        out=t2[:ns],
                        in_=Hp[:ns],
                        func=mybir.ActivationFunctionType.Relu,
                    )
                    nc.vector.tensor_scalar(
                        out=t2[:ns], in0=t2[:ns],
                        scalar1=float(1.0 - alpha), scalar2=None,
                        op0=mybir.AluOpType.mult,
                    )
                    t3 = outp.tile([P, hid], F32, tag="t3", name=f"t3_{b}")
                    nc.vector.tensor_scalar(
                        out=t3[:ns], in0=Hp[:ns],
                        scalar1=float(alpha), scalar2=None,
                        op0=mybir.AluOpType.mult,
                    )
                    nc.vector.tensor_tensor(
                        out=osl[:ns], in0=t2[:ns], in1=t3[:ns],
                        op=mybir.AluOpType.add,
                    )
                if slot == obat - 1 or b == bpc - 1:
                    b0 = b - slot
                    nc.sync.dma_start(
                        out=out_d.ap()[:, b0 * hid: (b + 1) * hid],
                        in_=os_[:, : (slot + 1) * hid],
                    )
    nc.compile()
    return nc


def _make_in_maps(x, weight, bias, idx16, dstl, uniq, dinv, kblk, owned,
                  ncores=NCORES):
    import ml_dtypes
    bf = ml_dtypes.bfloat16
    x = np.asarray(x, dtype=np.float32)
    w = np.asarray(weight, dtype=np.float32)
    n = x.shape[0]
    in_ch = x.shape[1]
    hid = w.shape[0]
    npc = n // ncores
    bpc = len(kblk)
    npc_pad = bpc * P
    gmax = max(kblk)
    tot = sum(kblk)

    xp = x * dinv[:, None]  # dinv[src] folded into features
    iota = np.tile(np.arange(P, dtype=np.float32), (P, gmax)).astype(bf)
    wts = {
        f"wt{h}": np.ascontiguousarray(
            w[:, h * P: (h + 1) * P].T.astype(bf))
        for h in range(in_ch // P)
    }
    bias_row = np.asarray(bias, dtype=np.float32).reshape(1, hid).astype(bf)
    in_maps = []
    for k in range(ncores):
        xtab = np.zeros((TAB, in_ch), bf)
        xtab[: len(uniq[k])] = xp[uniq[k]].astype(bf)
        xsp_ = np.zeros((npc_pad, in_ch), np.float32)
        xsp_[:npc] = xp[owned[k]]
        # [p, b*in_ch + c] = xp[b*128 + p, c]
        xs = np.ascontiguousarray(
            xsp_.reshape(bpc, P, in_ch).transpose(1, 0, 2)
            .reshape(P, bpc * in_ch)).astype(bf)
        dnv = np.zeros((P, bpc), np.float32)
        dcore = np.zeros(npc_pad, np.float32)
        dcore[:npc] = dinv[owned[k]]
        dnv[:, :] = dcore.reshape(bpc, P).T
        m = {
            "xtab": xtab,
            "idx16": np.ascontiguousarray(idx16[k]),
            "dstl": np.ascontiguousarray(dstl[k].astype(bf)),
            "iota": iota,
            "xself": xs,
            "dnv": dnv,
            "bias": bias_row,
            "ones": np.ones((1, P), bf),
            "idr": np.eye(P, dtype=bf),
        }
        m.update(wts)
        in_maps.append(m)
    return in_maps


# Results of the last kernel() call, for the test harness.
LAST_RESULTS = None


def unpack_out(arr, npc=NPC, hid=HID):
    """[128, bpc*hid] block-major device output -> [npc, hid] f32."""
    arr = np.asarray(arr)
    bpc = arr.shape[1] // hid
    full = arr.reshape(P, bpc, hid).transpose(1, 0, 2).reshape(bpc * P, hid)
    return full[:npc].astype(np.float32)


def kernel(x, edge_index, weight, bias, prelu_a):
    global LAST_RESULTS
    trace = os.environ.get("GCN_TRACE", "0") == "1"

    kblk, groups, idx16, dstl, uniq, dinv, owned = _preprocess(edge_index)
    alpha = float(np.asarray(prelu_a).ravel()[0])
    nc = _build_program(kblk, groups, alpha)
    in_maps = _make_in_maps(x, weight, bias, idx16, dstl, uniq, dinv, kblk,
                            owned)

    res = bass_utils.run_bass_kernel_spmd(
        nc, in_maps, core_ids=list(range(NCORES)), trace=trace
    )
    LAST_RESULTS = res
    dev = np.concatenate(
        [unpack_out(res.results[k]["out"]) for k in range(NCORES)], axis=0)
    out = np.empty_like(dev)
    out[np.concatenate(owned)] = dev
    return out
